# revision 1
# baseline (speedup 1.0000x reference)
"""Bootstrap-ensemble MLP (100 models, D=16 -> H=128 x5 -> mu/sigma heads)
on 8 Trainium2 NeuronCores.

Sharding: every core runs an identical SPMD program over 25 models x 8192
batch points (model axis split 4 ways x batch split 2 ways) -- perfectly
balanced.  All per-core weights are pre-arranged on the host into the exact
SBUF layouts the TensorEngine wants (lhsT = pre-transposed stationary
operand), so the device does no transposes at all.

Compute structure per core:
- fp32r matmuls (TF32-class precision, 4x the throughput of fp32 on the PE)
- models interleaved in groups of 4 so PE always has independent matmuls
  while ACT/DVE run another model's bias+ReLU (fused into one op each)
- layer-1 (K=16) matmuls of the 4 models in a group run concurrently in
  different 32-row groups of the PE array (tile_position row tiling)
- mu/sigma head matmuls of model pairs run concurrently in different column
  halves (tile_position col tiling), accumulating all 25 models into one
  [128, CH] PSUM tile via zero-padded per-model head weights; finished with
  Identity/Exp activations with the bias folded in.
"""

import os

import numpy as np

M = 100  # n_models
D = 16  # input_dim
H = 128  # hidden_dim
O = 1  # output_dim
NH = 4  # n_hidden
N = 16384  # batch of query points

NCORES = 8
MPC = 25  # models per core
NB = 4  # model blocks
NHALF = N // 2  # 8192 points per core
CH = 1024  # chunk of batch points processed at once
NCH = NHALF // CH  # 8 chunks
MM_N = 512  # matmul moving free dim (one PSUM bank of fp32)
NEV = (MPC + 1) // 2  # 13 even-index models (head half A)
NOD = MPC // 2  # 12 odd-index models (head half B)

_CACHE: dict = {}


def _build_module():
    import concourse.bacc as bacc
    import concourse.mybir as mybir
    import concourse.tile as tile

    f32 = mybir.dt.float32
    f32m = (
        mybir.dt.float32
        if os.environ.get("KERNEL_MM_FP32", "0") == "1"
        else mybir.dt.float32r
    )
    AF = mybir.ActivationFunctionType
    ALU = mybir.AluOpType

    nc = bacc.Bacc(
        "TRN2",
        target_bir_lowering=False,
        debug=False,
        num_devices=NCORES,
    )

    NBLK = (MPC + 3) // 4  # 7 row-tiling blocks of up to 4 models
    xt_d = nc.dram_tensor("xt", [128, NHALF], f32m, kind="ExternalInput")
    w1t_d = nc.dram_tensor("w1t", [128, NBLK * H], f32m, kind="ExternalInput")
    wht_d = nc.dram_tensor("wht", [H, MPC * NH * H], f32m, kind="ExternalInput")
    whd_d = nc.dram_tensor("whd", [H, MPC * 64], f32m, kind="ExternalInput")
    b1_d = nc.dram_tensor("b1", [H, MPC], f32, kind="ExternalInput")
    bh_d = nc.dram_tensor("bh", [H, MPC * NH], f32, kind="ExternalInput")
    bhd_d = nc.dram_tensor("bhd", [64, 1], f32, kind="ExternalInput")
    mu_d = nc.dram_tensor("mu", [MPC, NHALF], f32, kind="ExternalOutput")
    sig_d = nc.dram_tensor("sig", [MPC, NHALF], f32, kind="ExternalOutput")

    with tile.TileContext(nc) as tc:
        with (
            tc.tile_pool(name="const", bufs=1) as const,
            tc.tile_pool(name="hpool", bufs=14) as hpool,
            tc.tile_pool(name="opool", bufs=2) as opool,
            tc.tile_pool(name="mmpsum", bufs=3, space="PSUM") as mmpsum,
            tc.tile_pool(name="hdpsum", bufs=1, space="PSUM") as hdpsum,
        ):
            xt = const.tile([128, NHALF], f32m)
            w1t = const.tile([128, NBLK * H], f32m)
            wht = const.tile([H, MPC * NH * H], f32m)
            whd = const.tile([H, MPC * 64], f32m)
            b1 = const.tile([H, MPC], f32)
            bh = const.tile([H, MPC * NH], f32)
            bhd = const.tile([64, 1], f32)

            nc.sync.dma_start(w1t[:], w1t_d[:])
            nc.sync.dma_start(b1[:], b1_d[:])
            nc.sync.dma_start(bh[:], bh_d[:])
            nc.sync.dma_start(bhd[:], bhd_d[:])
            nc.sync.dma_start(whd[:], whd_d[:])
            # chunked so the first models' matmuls don't wait on the full blob
            for m in range(MPC):
                s = m * NH * H
                nc.sync.dma_start(wht[:, s : s + NH * H], wht_d[:, s : s + NH * H])
            for nt in range(NCH):
                s = nt * CH
                nc.sync.dma_start(xt[:, s : s + CH], xt_d[:, s : s + CH])

            def relu_act(dst, src, bias_ap):
                nc.scalar.activation(dst, src, AF.Relu, bias=bias_ap)

            def relu_dve(dst, src, bias_ap):
                nc.vector.tensor_scalar(dst, src, bias_ap, 0.0, ALU.add, ALU.max)

            groups = [list(range(b * 4, min(b * 4 + 4, MPC))) for b in range(NBLK)]
            units = [(nt, bi) for nt in range(NCH) for bi in range(len(groups))]
            h_l1 = {}

            def emit_l1(nt, bi):
                c0 = nt * CH
                grp = groups[bi]
                for m in grp:
                    h_l1[(nt, m)] = hpool.tile([128, CH], f32m, tag="h", name="h")
                for s in range(0, CH, MM_N):
                    tiles = [
                        mmpsum.tile([128, CH], f32, tag="mm", name=f"l1ps{k}")
                        for k in range((len(grp) + 1) // 2)
                    ]
                    for j, m in enumerate(grp):
                        t = tiles[j // 2]
                        reg = t[:, (j % 2) * MM_N : (j % 2 + 1) * MM_N]
                        nc.tensor.matmul(
                            reg,
                            w1t[32 * j : 32 * j + D, bi * H : (bi + 1) * H],
                            xt[32 * j : 32 * j + D, c0 + s : c0 + s + MM_N],
                            start=True,
                            stop=True,
                            tile_position=(32 * j, 0),
                        )
                    for j, m in enumerate(grp):
                        t = tiles[j // 2]
                        reg = t[:, (j % 2) * MM_N : (j % 2 + 1) * MM_N]
                        rl = relu_act if (j + s // MM_N) % 2 == 0 else relu_dve
                        rl(h_l1[(nt, m)][:, s : s + MM_N], reg, b1[:, m : m + 1])

            emit_l1(*units[0])
            hp = None
            pending_ep = []

            def emit_epilogue(hp_t, c0_t):
                mu_t = opool.tile([MPC, CH], f32, tag="mu")
                sig_t = opool.tile([MPC, CH], f32, tag="sig")
                nc.scalar.activation(
                    mu_t[:], hp_t[0:MPC, :], AF.Identity, bias=bhd[0:MPC, :]
                )
                nc.scalar.activation(
                    sig_t[:], hp_t[32 : 32 + MPC, :], AF.Exp,
                    bias=bhd[32 : 32 + MPC, :],
                )
                nc.sync.dma_start(mu_d[:, c0_t : c0_t + CH], mu_t[:])
                nc.sync.dma_start(sig_d[:, c0_t : c0_t + CH], sig_t[:])

            for u, (nt, bi) in enumerate(units):
                c0 = nt * CH
                grp = groups[bi]
                if bi == 0:
                    hp = hdpsum.tile([64, CH], f32, tag="hp", name="hp")
                hcur = {m: h_l1.pop((nt, m)) for m in grp}
                # hidden layers, interleaved across the group
                for i in range(NH):
                    for m in grp:
                        ps = mmpsum.tile([128, CH], f32, tag="mm")
                        lhsh = wht[:, (m * NH + i) * H : (m * NH + i + 1) * H]
                        for s in range(0, CH, MM_N):
                            nc.tensor.matmul(
                                ps[:, s : s + MM_N],
                                lhsh,
                                hcur[m][:, s : s + MM_N],
                                start=True,
                                stop=True,
                            )
                        hn = hpool.tile([128, CH], f32m, tag="h")
                        bias_ap = bh[:, m * NH + i : m * NH + i + 1]
                        # alternate engines per (model, layer); every 8th model
                        # gives one extra layer to ACT to balance totals
                        on_act = (m + i) % 2 == 0 or (m % 8 == 0 and i == 1)
                        rl = relu_act if on_act else relu_dve
                        rl(hn[:], ps[:], bias_ap)
                        hcur[m] = hn
                    if i == 0 and pending_ep:
                        # previous chunk's mu/sigma finish, emitted here so the
                        # boundary ReLUs aren't queued behind them
                        emit_epilogue(*pending_ep.pop())
                    if i == NH - 2 and u + 1 < len(units):
                        # prefetch next unit's layer-1 while this unit finishes
                        emit_l1(*units[u + 1])
                # heads: accumulate all 25 models into one [64, CH] psum
                for m in grp:
                    lhshd = whd[:, m * 64 : (m + 1) * 64]
                    for s in range(0, CH, MM_N):
                        nc.tensor.matmul(
                            hp[:, s : s + MM_N],
                            lhshd,
                            hcur[m][:, s : s + MM_N],
                            start=(m == 0),
                            stop=(m == MPC - 1),
                        )
                if bi == len(groups) - 1:
                    pending_ep.append((hp, c0))
            while pending_ep:
                emit_epilogue(*pending_ep.pop())

    nc.compile()
    return nc


def _get_module():
    if "nc" not in _CACHE:
        _CACHE["nc"] = _build_module()
    return _CACHE["nc"]


def _shard_inputs(x, W1, b1, Wh, bh, Wmu, bmu, Wsig, bsig):
    """Build the per-core input maps (host-side layout prep)."""
    NBLK = (MPC + 3) // 4
    in_maps = []
    for c in range(NCORES):
        mb, half = c % NB, c // NB
        ms = slice(MPC * mb, MPC * (mb + 1))
        xh = x[NHALF * half : NHALF * (half + 1), :]  # [8192, 16]
        xtr = np.ascontiguousarray(xh.T)  # [16, 8192]
        xt_full = np.zeros((128, NHALF), dtype=np.float32)
        for rep in range(4):  # replicas at partition 0/32/64/96 for row tiling
            xt_full[32 * rep : 32 * rep + D, :] = xtr

        w1 = W1[ms]  # [25, 128, 16]
        w1t = np.zeros((128, NBLK * H), dtype=np.float32)
        for m in range(MPC):
            b, g = m // 4, m % 4
            w1t[32 * g : 32 * g + D, b * H : (b + 1) * H] = w1[m].T

        wh = Wh[ms]  # [25, 4, 128, 128] (out, in)
        wht = np.ascontiguousarray(
            wh.transpose(3, 0, 1, 2).reshape(H, MPC * NH * H)
        )  # [h_in, (m, i, h_out)]

        whd = np.zeros((H, MPC * 64), dtype=np.float32)
        for m in range(MPC):
            base = m * 64
            whd[:, base + m] = Wmu[ms][m, 0, :]
            whd[:, base + 32 + m] = Wsig[ms][m, 0, :]

        b1p = np.ascontiguousarray(b1[ms].T)  # [128, 25]
        bhp = np.ascontiguousarray(
            bh[ms].transpose(2, 0, 1).reshape(H, MPC * NH)
        )  # [128, (m, i)]
        bhdp = np.zeros((64, 1), dtype=np.float32)
        bhdp[0:MPC, 0] = bmu[ms][:, 0]
        bhdp[32 : 32 + MPC, 0] = bsig[ms][:, 0]

        in_maps.append(
            {
                "xt": xt_full,
                "w1t": w1t,
                "wht": wht,
                "whd": whd,
                "b1": b1p,
                "bh": bhp,
                "bhd": bhdp,
            }
        )
    return in_maps


def _run(in_maps, trace=False):
    from concourse.bass_utils import run_bass_kernel_spmd

    nc = _get_module()
    return run_bass_kernel_spmd(
        nc, in_maps, list(range(NCORES)), trace=trace
    )


def kernel(x, W1, b1, Wh, bh, Wmu, bmu, Wsig, bsig):
    args = [
        np.ascontiguousarray(np.asarray(a, dtype=np.float32))
        for a in (x, W1, b1, Wh, bh, Wmu, bmu, Wsig, bsig)
    ]
    in_maps = _shard_inputs(*args)
    res = _run(in_maps, trace=bool(int(os.environ.get("KERNEL_TRACE", "0"))))
    _CACHE["last_results"] = res

    mu = np.empty((M, N), dtype=np.float32)
    sig = np.empty((M, N), dtype=np.float32)
    for c in range(NCORES):
        mb, half = c % NB, c // NB
        m0 = MPC * mb
        ns = slice(NHALF * half, NHALF * (half + 1))
        r = res.results[c]
        mu[m0 : m0 + MPC, ns] = r["mu"]
        sig[m0 : m0 + MPC, ns] = r["sig"]
    return (mu.reshape(M, N, O), sig.reshape(M, N, O))



# revision 6
# speedup vs baseline: 1.1071x; 1.1071x over previous
"""Bootstrap-ensemble MLP (100 models, D=16 -> H=128 x5 -> mu/sigma heads)
on 8 Trainium2 NeuronCores.

Sharding: every core runs an identical SPMD program over 25 models x 8192
batch points (model axis split 4 ways x batch split 2 ways) -- perfectly
balanced.  All per-core weights are pre-arranged on the host into the exact
SBUF layouts the TensorEngine wants (lhsT = pre-transposed stationary
operand), so the device does no transposes at all.

Compute structure per core:
- bf16 matmul operands (1 col/cycle on the PE -- 2x the fp32r rate -- and
  fast weight loads), fp32 PSUM accumulation
- models interleaved in groups of 4 so PE always has independent matmuls
  while ACT/DVE run another model's bias+ReLU (fused into one op each)
- layer-1 (K=16) matmuls of the 4 models in a group run concurrently in
  different 32-row groups of the PE array (tile_position row tiling)
- mu/sigma head matmuls accumulate all 25 models into one [64, CH] PSUM
  tile via zero-padded per-model head weights; finished with a DVE bias-add
  (mu) and an ACT Exp (sigma) with the bias folded in
- bias+ReLU ops are assigned to ACT vs DVE by greedy compile-time load
  balancing using the measured cost models of both engines
"""

import os

import numpy as np

M = 100  # n_models
D = 16  # input_dim
H = 128  # hidden_dim
O = 1  # output_dim
NH = 4  # n_hidden
N = 16384  # batch of query points

NCORES = 8
MPC = 25  # models per core
NB = 4  # model blocks
NHALF = N // 2  # 8192 points per core
CH = 1024  # chunk of batch points processed at once
NCH = NHALF // CH  # 8 chunks
MM_N = 512  # matmul moving free dim (one PSUM bank of fp32)

_CACHE: dict = {}


def _build_module():
    import concourse.bacc as bacc
    import concourse.mybir as mybir
    import concourse.tile as tile

    f32 = mybir.dt.float32
    mmdt = os.environ.get("KERNEL_MM_DTYPE", "bf16")
    f32m = {
        "bf16": mybir.dt.bfloat16,
        "fp16": mybir.dt.float16,
        "fp32r": mybir.dt.float32r,
        "fp32": mybir.dt.float32,
    }[mmdt]
    AF = mybir.ActivationFunctionType
    ALU = mybir.AluOpType

    nc = bacc.Bacc(
        "TRN2",
        target_bir_lowering=False,
        debug=False,
        num_devices=NCORES,
    )

    NBLK = (MPC + 3) // 4  # 7 row-tiling blocks of up to 4 models
    xt_d = nc.dram_tensor("xt", [128, NHALF], f32m, kind="ExternalInput")
    w1t_d = nc.dram_tensor("w1t", [128, NBLK * H], f32m, kind="ExternalInput")
    wht_d = nc.dram_tensor("wht", [H, MPC * NH * H], f32m, kind="ExternalInput")
    whd_d = nc.dram_tensor("whd", [H, MPC * 64], f32m, kind="ExternalInput")
    b1_d = nc.dram_tensor("b1", [H, MPC], f32, kind="ExternalInput")
    bh_d = nc.dram_tensor("bh", [H, MPC * NH], f32, kind="ExternalInput")
    bhd_d = nc.dram_tensor("bhd", [64, 1], f32, kind="ExternalInput")
    mu_d = nc.dram_tensor("mu", [MPC, NHALF], f32, kind="ExternalOutput")
    sig_d = nc.dram_tensor("sig", [MPC, NHALF], f32, kind="ExternalOutput")

    # compile-time engine load balancing (ns, measured cost models)
    eng_load = {"act": 0.0, "dve": 0.0}

    with tile.TileContext(nc) as tc:
        with (
            tc.tile_pool(name="const", bufs=1) as const,
            tc.tile_pool(name="hpool", bufs=14) as hpool,
            tc.tile_pool(name="opool", bufs=2) as opool,
            tc.tile_pool(name="mmpsum", bufs=3, space="PSUM") as mmpsum,
            tc.tile_pool(name="hdpsum", bufs=1, space="PSUM") as hdpsum,
        ):
            xt = const.tile([128, NHALF], f32m)
            w1t = const.tile([128, NBLK * H], f32m)
            wht = const.tile([H, MPC * NH * H], f32m)
            whd = const.tile([H, MPC * 64], f32m)
            b1 = const.tile([H, MPC], f32)
            bh = const.tile([H, MPC * NH], f32)
            bhd = const.tile([64, 1], f32)

            nc.sync.dma_start(w1t[:], w1t_d[:])
            nc.sync.dma_start(b1[:], b1_d[:])
            nc.sync.dma_start(bh[:], bh_d[:])
            nc.sync.dma_start(bhd[:], bhd_d[:])
            nc.sync.dma_start(whd[:], whd_d[:])
            # chunked so the first models' matmuls don't wait on the full blob
            for m in range(MPC):
                s = m * NH * H
                nc.sync.dma_start(wht[:, s : s + NH * H], wht_d[:, s : s + NH * H])
            for nt in range(NCH):
                s = nt * CH
                nc.sync.dma_start(xt[:, s : s + CH], xt_d[:, s : s + CH])

            def relu(dst, src, bias_ap, fd):
                # pick the engine that would finish this op sooner
                # (constants fit from HW profile: ACT=(FD+311)/1.2,
                #  DVE=(FD+207)/0.96)
                c_act = (fd + 311) / 1.2
                c_dve = (fd + 207) / 0.96
                if eng_load["act"] + c_act <= eng_load["dve"] + c_dve:
                    eng_load["act"] += c_act
                    nc.scalar.activation(dst, src, AF.Relu, bias=bias_ap)
                else:
                    eng_load["dve"] += c_dve
                    nc.vector.tensor_scalar(
                        dst, src, bias_ap, 0.0, ALU.add, ALU.max
                    )

            groups = [list(range(b * 4, min(b * 4 + 4, MPC))) for b in range(NBLK)]
            units = [(nt, bi) for nt in range(NCH) for bi in range(len(groups))]
            h_l1 = {}

            def emit_l1(nt, bi):
                c0 = nt * CH
                grp = groups[bi]
                for m in grp:
                    h_l1[(nt, m)] = hpool.tile([128, CH], f32m, tag="h", name="h")
                for s in range(0, CH, MM_N):
                    tiles = [
                        mmpsum.tile([128, CH], f32, tag="mm", name=f"l1ps{k}")
                        for k in range((len(grp) + 1) // 2)
                    ]
                    for j, m in enumerate(grp):
                        t = tiles[j // 2]
                        reg = t[:, (j % 2) * MM_N : (j % 2 + 1) * MM_N]
                        nc.tensor.matmul(
                            reg,
                            w1t[32 * j : 32 * j + D, bi * H : (bi + 1) * H],
                            xt[32 * j : 32 * j + D, c0 + s : c0 + s + MM_N],
                            start=True,
                            stop=True,
                            tile_position=(32 * j, 0),
                        )
                    for j, m in enumerate(grp):
                        t = tiles[j // 2]
                        reg = t[:, (j % 2) * MM_N : (j % 2 + 1) * MM_N]
                        relu(h_l1[(nt, m)][:, s : s + MM_N], reg, b1[:, m : m + 1], MM_N)

            emit_l1(*units[0])
            hp = None
            pending_ep = []

            def emit_epilogue(hp_t, c0_t):
                mu_t = opool.tile([MPC, CH], f32, tag="mu")
                sig_t = opool.tile([MPC, CH], f32, tag="sig")
                nc.vector.tensor_scalar_add(mu_t[:], hp_t[0:MPC, :], bhd[0:MPC, :])
                eng_load["dve"] += (CH + 120) / 0.96
                nc.scalar.activation(
                    sig_t[:], hp_t[32 : 32 + MPC, :], AF.Exp,
                    bias=bhd[32 : 32 + MPC, :],
                )
                eng_load["act"] += (CH + 172) / 1.2
                nc.sync.dma_start(mu_d[:, c0_t : c0_t + CH], mu_t[:])
                nc.sync.dma_start(sig_d[:, c0_t : c0_t + CH], sig_t[:])

            for u, (nt, bi) in enumerate(units):
                c0 = nt * CH
                grp = groups[bi]
                if bi == 0:
                    hp = hdpsum.tile([64, CH], f32, tag="hp", name="hp")
                hcur = {m: h_l1.pop((nt, m)) for m in grp}
                # hidden layers, interleaved across the group
                for i in range(NH):
                    for m in grp:
                        ps = mmpsum.tile([128, CH], f32, tag="mm")
                        lhsh = wht[:, (m * NH + i) * H : (m * NH + i + 1) * H]
                        for s in range(0, CH, MM_N):
                            nc.tensor.matmul(
                                ps[:, s : s + MM_N],
                                lhsh,
                                hcur[m][:, s : s + MM_N],
                                start=True,
                                stop=True,
                            )
                        hn = hpool.tile([128, CH], f32m, tag="h")
                        bias_ap = bh[:, m * NH + i : m * NH + i + 1]
                        relu(hn[:], ps[:], bias_ap, CH)
                        hcur[m] = hn
                    if i == 0 and pending_ep:
                        # previous chunk's mu/sigma finish, emitted here so the
                        # boundary ReLUs aren't queued behind them
                        emit_epilogue(*pending_ep.pop())
                    if i == NH - 2 and u + 1 < len(units):
                        # prefetch next unit's layer-1 while this unit finishes
                        emit_l1(*units[u + 1])
                # heads: accumulate all 25 models into one [64, CH] psum
                for m in grp:
                    lhshd = whd[:, m * 64 : (m + 1) * 64]
                    for s in range(0, CH, MM_N):
                        nc.tensor.matmul(
                            hp[:, s : s + MM_N],
                            lhshd,
                            hcur[m][:, s : s + MM_N],
                            start=(m == 0),
                            stop=(m == MPC - 1),
                        )
                if bi == len(groups) - 1:
                    pending_ep.append((hp, c0))
            while pending_ep:
                emit_epilogue(*pending_ep.pop())

    _dedupe_ldweights(nc, mybir)
    nc.compile()
    return nc


def _dedupe_ldweights(nc, mybir):
    """Delete LDWEIGHTS whose exact weights are already resident in the same
    PE-array region (the Tile lowering re-emits one per matmul).  Weights
    persist in the array across matmuls, so back-to-back matmuls on the same
    stationary operand only need the first load.  Region tracking handles
    row/col-tiled partial loads (an overlapping load invalidates)."""
    removed = 0
    for blk in nc.main_func.blocks:
        loaded: dict = {}
        out = []
        for inst in blk.instructions:
            if isinstance(inst, mybir.InstLdweights):
                w = inst.ins[0]
                key = (
                    getattr(w, "memref", None),
                    w.offset,
                    str(w.ap),
                    str(w.dtype),
                    inst.tile_position,
                    inst.tile_size,
                    inst.perf_mode,
                    inst.is_transpose,
                )
                tp = inst.tile_position or (0, 0)
                ts = inst.tile_size or (128, 128)
                region = (tp[0], tp[0] + ts[0], tp[1], tp[1] + ts[1])
                si = inst.sync_info
                no_sync = si is None or (not si.on_wait and not si.on_update)
                if no_sync and loaded.get(region) == key:
                    removed += 1
                    continue
                for r in list(loaded):
                    if not (
                        r[1] <= region[0]
                        or region[1] <= r[0]
                        or r[3] <= region[2]
                        or region[3] <= r[2]
                    ):
                        del loaded[r]
                loaded[region] = key
            out.append(inst)
        blk.instructions[:] = out
    return removed


def _get_module():
    if "nc" not in _CACHE:
        _CACHE["nc"] = _build_module()
    return _CACHE["nc"]


def _mm_np_dtype():
    mmdt = os.environ.get("KERNEL_MM_DTYPE", "bf16")
    if mmdt == "bf16":
        import ml_dtypes

        return ml_dtypes.bfloat16
    return np.float32


def _shard_inputs(x, W1, b1, Wh, bh, Wmu, bmu, Wsig, bsig):
    """Build the per-core input maps (host-side layout prep)."""
    NBLK = (MPC + 3) // 4
    mdt = _mm_np_dtype()
    in_maps = []
    for c in range(NCORES):
        mb, half = c % NB, c // NB
        ms = slice(MPC * mb, MPC * (mb + 1))
        xh = x[NHALF * half : NHALF * (half + 1), :]  # [8192, 16]
        xtr = np.ascontiguousarray(xh.T)  # [16, 8192]
        xt_full = np.zeros((128, NHALF), dtype=np.float32)
        for rep in range(4):  # replicas at partition 0/32/64/96 for row tiling
            xt_full[32 * rep : 32 * rep + D, :] = xtr

        w1 = W1[ms]  # [25, 128, 16]
        w1t = np.zeros((128, NBLK * H), dtype=np.float32)
        for m in range(MPC):
            b, g = m // 4, m % 4
            w1t[32 * g : 32 * g + D, b * H : (b + 1) * H] = w1[m].T

        wh = Wh[ms]  # [25, 4, 128, 128] (out, in)
        wht = np.ascontiguousarray(
            wh.transpose(3, 0, 1, 2).reshape(H, MPC * NH * H)
        )  # [h_in, (m, i, h_out)]

        whd = np.zeros((H, MPC * 64), dtype=np.float32)
        for m in range(MPC):
            base = m * 64
            whd[:, base + m] = Wmu[ms][m, 0, :]
            whd[:, base + 32 + m] = Wsig[ms][m, 0, :]

        b1p = np.ascontiguousarray(b1[ms].T)  # [128, 25]
        bhp = np.ascontiguousarray(
            bh[ms].transpose(2, 0, 1).reshape(H, MPC * NH)
        )  # [128, (m, i)]
        bhdp = np.zeros((64, 1), dtype=np.float32)
        bhdp[0:MPC, 0] = bmu[ms][:, 0]
        bhdp[32 : 32 + MPC, 0] = bsig[ms][:, 0]

        in_maps.append(
            {
                "xt": xt_full.astype(mdt),
                "w1t": w1t.astype(mdt),
                "wht": wht.astype(mdt),
                "whd": whd.astype(mdt),
                "b1": b1p,
                "bh": bhp,
                "bhd": bhdp,
            }
        )
    return in_maps


def _run(in_maps, trace=False):
    from concourse.bass_utils import run_bass_kernel_spmd

    nc = _get_module()
    return run_bass_kernel_spmd(
        nc, in_maps, list(range(NCORES)), trace=trace
    )


def kernel(x, W1, b1, Wh, bh, Wmu, bmu, Wsig, bsig):
    args = [
        np.ascontiguousarray(np.asarray(a, dtype=np.float32))
        for a in (x, W1, b1, Wh, bh, Wmu, bmu, Wsig, bsig)
    ]
    in_maps = _shard_inputs(*args)
    res = _run(in_maps, trace=bool(int(os.environ.get("KERNEL_TRACE", "0"))))
    _CACHE["last_results"] = res

    mu = np.empty((M, N), dtype=np.float32)
    sig = np.empty((M, N), dtype=np.float32)
    for c in range(NCORES):
        mb, half = c % NB, c // NB
        m0 = MPC * mb
        ns = slice(NHALF * half, NHALF * (half + 1))
        r = res.results[c]
        mu[m0 : m0 + MPC, ns] = r["mu"]
        sig[m0 : m0 + MPC, ns] = r["sig"]
    return (mu.reshape(M, N, O), sig.reshape(M, N, O))


# revision 9
# speedup vs baseline: 1.2982x; 1.1726x over previous
"""Bootstrap-ensemble MLP (100 models, D=16 -> H=128 x5 -> mu/sigma heads)
on 8 Trainium2 NeuronCores.

Sharding: every core runs an identical SPMD program over 25 models x 8192
batch points (model axis split 4 ways x batch split 2 ways) -- perfectly
balanced.  All per-core weights are pre-arranged on the host into the exact
SBUF layouts the TensorEngine wants (lhsT = pre-transposed stationary
operand), so the device does no transposes at all.

Compute structure per core:
- bf16 matmul operands (fp32 PSUM accumulation), biases fp32
- models interleaved in groups of 4 so PE always has independent matmuls
  while ACT/DVE run another model's bias+ReLU (fused into one op each)
- layer-1 (K=17, bias folded in as an extra contraction row against a
  constant-one row of x) matmuls of the 4 models in a group run concurrently
  in different 32-row groups of the PE array (tile_position row tiling)
- 4 rotating [128, CH] PSUM tiles (full 8 banks): the mu/sigma head matmuls
  run as a deferred per-chunk streak (from saved layer-4 h tiles) into a
  transiently-held pool tile, col-tiled 2x so even/odd models' head matmuls
  run concurrently in different column halves of the PE array
- a post-schedule pass deletes LDWEIGHTS instructions whose exact weights
  are already resident in the targeted PE-array region (the Tile lowering
  re-emits one per matmul; weights persist across matmuls)
- bias+ReLU ops are assigned to ACT vs DVE by greedy compile-time load
  balancing using cost models fit from HW profiles
"""

import os

import numpy as np

M = 100  # n_models
D = 16  # input_dim
H = 128  # hidden_dim
O = 1  # output_dim
NH = 4  # n_hidden
N = 16384  # batch of query points

NCORES = 8
MPC = 25  # models per core
NB = 4  # model blocks
NHALF = N // 2  # 8192 points per core
CH = 1024  # chunk of batch points processed at once
NCH = NHALF // CH  # 8 chunks
MM_N = 512  # matmul moving free dim (one PSUM bank of fp32)
NEV = (MPC + 1) // 2  # 13 even-index models (head col-group 0)
NOD = MPC // 2  # 12 odd-index models (head col-group 1)

_CACHE: dict = {}


def _build_module():
    import concourse.bacc as bacc
    import concourse.mybir as mybir
    import concourse.tile as tile

    f32 = mybir.dt.float32
    mmdt = os.environ.get("KERNEL_MM_DTYPE", "bf16")
    f32m = {
        "bf16": mybir.dt.bfloat16,
        "fp16": mybir.dt.float16,
        "fp32r": mybir.dt.float32r,
        "fp32": mybir.dt.float32,
    }[mmdt]
    AF = mybir.ActivationFunctionType
    ALU = mybir.AluOpType

    nc = bacc.Bacc(
        "TRN2",
        target_bir_lowering=False,
        debug=False,
        num_devices=NCORES,
    )

    NBLK = (MPC + 3) // 4  # 7 row-tiling blocks of up to 4 models
    DK = D + 1  # L1 contraction rows incl folded bias
    xt_d = nc.dram_tensor("xt", [128, NHALF], f32m, kind="ExternalInput")
    w1t_d = nc.dram_tensor("w1t", [128, NBLK * H], f32m, kind="ExternalInput")
    wht_d = nc.dram_tensor("wht", [H, MPC * NH * H], f32m, kind="ExternalInput")
    whd_d = nc.dram_tensor("whd", [H, MPC * 64], f32m, kind="ExternalInput")
    bh_d = nc.dram_tensor("bh", [H, MPC * NH], f32, kind="ExternalInput")
    bhd_d = nc.dram_tensor("bhd", [128, 1], f32, kind="ExternalInput")
    mu_d = nc.dram_tensor("mu", [MPC, NHALF], f32, kind="ExternalOutput")
    sig_d = nc.dram_tensor("sig", [MPC, NHALF], f32, kind="ExternalOutput")

    # compile-time engine load balancing (ns, cost models fit from profiles)
    eng_load = {"act": 0.0, "dve": 0.0}

    with tile.TileContext(nc) as tc:
        with (
            tc.tile_pool(name="const", bufs=1) as const,
            tc.tile_pool(name="hpool", bufs=44) as hpool,
            tc.tile_pool(name="opool", bufs=2) as opool,
            tc.tile_pool(name="mmpsum", bufs=4, space="PSUM") as mmpsum,
        ):
            xt = const.tile([128, NHALF], f32m)
            w1t = const.tile([128, NBLK * H], f32m)
            wht = const.tile([H, MPC * NH * H], f32m)
            whd = const.tile([H, MPC * 64], f32m)
            bh = const.tile([H, MPC * NH], f32)
            bhd = const.tile([128, 1], f32)

            nc.sync.dma_start(w1t[:], w1t_d[:])
            nc.sync.dma_start(bh[:], bh_d[:])
            nc.sync.dma_start(bhd[:], bhd_d[:])
            nc.sync.dma_start(whd[:], whd_d[:])
            # chunked so the first models' matmuls don't wait on the full blob
            for m in range(MPC):
                s = m * NH * H
                nc.sync.dma_start(wht[:, s : s + NH * H], wht_d[:, s : s + NH * H])
            for nt in range(NCH):
                s = nt * CH
                nc.sync.dma_start(xt[:, s : s + CH], xt_d[:, s : s + CH])

            def relu(dst, src, bias_ap, fd):
                # pick the engine that would finish this op sooner
                # (constants fit from HW profile: ACT=(FD+311)/1.2,
                #  DVE=(FD+207)/0.96)
                c_act = (fd + 311) / 1.2
                c_dve = (fd + 207) / 0.96
                if eng_load["act"] + c_act <= eng_load["dve"] + c_dve:
                    eng_load["act"] += c_act
                    if bias_ap is None:
                        nc.scalar.activation(dst, src, AF.Relu)
                    else:
                        nc.scalar.activation(dst, src, AF.Relu, bias=bias_ap)
                else:
                    eng_load["dve"] += c_dve
                    if bias_ap is None:
                        nc.vector.tensor_scalar_max(dst, src, 0.0)
                    else:
                        nc.vector.tensor_scalar(
                            dst, src, bias_ap, 0.0, ALU.add, ALU.max
                        )

            groups = [list(range(b * 4, min(b * 4 + 4, MPC))) for b in range(NBLK)]
            units = [(nt, bi) for nt in range(NCH) for bi in range(len(groups))]
            h_l1 = {}
            h_fin = {}  # (nt, m) -> final-layer h tile awaiting head matmuls

            def emit_l1(nt, bi):
                c0 = nt * CH
                grp = groups[bi]
                for m in grp:
                    h_l1[(nt, m)] = hpool.tile([128, CH], f32m, tag="h", name="h")
                for s in range(0, CH, MM_N):
                    tiles = [
                        mmpsum.tile([128, CH], f32, tag="mm", name=f"l1ps{k}")
                        for k in range((len(grp) + 1) // 2)
                    ]
                    for j, m in enumerate(grp):
                        t = tiles[j // 2]
                        reg = t[:, (j % 2) * MM_N : (j % 2 + 1) * MM_N]
                        nc.tensor.matmul(
                            reg,
                            w1t[32 * j : 32 * j + DK, bi * H : (bi + 1) * H],
                            xt[32 * j : 32 * j + DK, c0 + s : c0 + s + MM_N],
                            start=True,
                            stop=True,
                            tile_position=(32 * j, 0),
                        )
                    for j, m in enumerate(grp):
                        t = tiles[j // 2]
                        reg = t[:, (j % 2) * MM_N : (j % 2 + 1) * MM_N]
                        relu(h_l1[(nt, m)][:, s : s + MM_N], reg, None, MM_N)

            def emit_heads(nt):
                """Head matmuls for all 25 models of chunk nt, col-tiled 2x:
                even models stream through array cols 0-31 into hp_e, odd
                models through cols 32-63 into hp_o (separate PSUM banks ->
                independent accumulation groups, concurrent on the PE), then
                the mu (DVE bias-add) / sigma (ACT exp) epilogue + DMA out."""
                c0 = nt * CH
                hp_e = mmpsum.tile([128, CH], f32, tag="mm", name="hpe")
                hp_o = mmpsum.tile([128, CH], f32, tag="mm", name="hpo")
                for m in range(MPC):
                    g = m % 2
                    hp = hp_e if g == 0 else hp_o
                    lhshd = whd[:, m * 64 : (m + 1) * 64]
                    hf = h_fin.pop((nt, m))
                    for s in range(0, CH, MM_N):
                        nc.tensor.matmul(
                            hp[64 * g : 64 * g + 64, s : s + MM_N],
                            lhshd,
                            hf[:, s : s + MM_N],
                            start=(m <= 1),
                            stop=(m >= MPC - 2),
                            tile_position=(0, 64 * g),
                            skip_group_check=True,
                        )
                mu_t = opool.tile([128, CH], f32, tag="mu")
                sig_t = opool.tile([128, CH], f32, tag="sig")
                nc.vector.tensor_scalar_add(
                    mu_t[0:NEV, :], hp_e[0:NEV, :], bhd[0:NEV, :]
                )
                nc.vector.tensor_scalar_add(
                    mu_t[64 : 64 + NOD, :], hp_o[64 : 64 + NOD, :],
                    bhd[64 : 64 + NOD, :],
                )
                eng_load["dve"] += 2 * (CH + 207) / 0.96
                nc.scalar.activation(
                    sig_t[32 : 32 + NEV, :], hp_e[32 : 32 + NEV, :], AF.Exp,
                    bias=bhd[32 : 32 + NEV, :],
                )
                nc.scalar.activation(
                    sig_t[96 : 96 + NOD, :], hp_o[96 : 96 + NOD, :], AF.Exp,
                    bias=bhd[96 : 96 + NOD, :],
                )
                eng_load["act"] += 2 * (CH + 311) / 1.2
                nc.sync.dma_start(mu_d[0:NEV, c0 : c0 + CH], mu_t[0:NEV, :])
                nc.sync.dma_start(
                    mu_d[NEV:MPC, c0 : c0 + CH], mu_t[64 : 64 + NOD, :]
                )
                nc.sync.dma_start(
                    sig_d[0:NEV, c0 : c0 + CH], sig_t[32 : 32 + NEV, :]
                )
                nc.sync.dma_start(
                    sig_d[NEV:MPC, c0 : c0 + CH], sig_t[96 : 96 + NOD, :]
                )

            emit_l1(*units[0])
            pending_heads = []

            for u, (nt, bi) in enumerate(units):
                grp = groups[bi]
                hcur = {m: h_l1.pop((nt, m)) for m in grp}
                # hidden layers, interleaved across the group
                for i in range(NH):
                    for m in grp:
                        ps = mmpsum.tile([128, CH], f32, tag="mm")
                        lhsh = wht[:, (m * NH + i) * H : (m * NH + i + 1) * H]
                        for s in range(0, CH, MM_N):
                            nc.tensor.matmul(
                                ps[:, s : s + MM_N],
                                lhsh,
                                hcur[m][:, s : s + MM_N],
                                start=True,
                                stop=True,
                            )
                        hn = hpool.tile([128, CH], f32m, tag="h")
                        bias_ap = bh[:, m * NH + i : m * NH + i + 1]
                        relu(hn[:], ps[:], bias_ap, CH)
                        hcur[m] = hn
                    if i == 1 and bi == 2 and pending_heads:
                        # previous chunk's head streak, deferred into this
                        # chunk so ACT/DVE have relu work while PE streams it
                        emit_heads(pending_heads.pop())
                    if i == NH - 2 and u + 1 < len(units):
                        # prefetch next unit's layer-1 while this unit finishes
                        emit_l1(*units[u + 1])
                for m in grp:
                    h_fin[(nt, m)] = hcur[m]
                if bi == len(groups) - 1:
                    pending_heads.append(nt)
            while pending_heads:
                emit_heads(pending_heads.pop(0))

    _dedupe_ldweights(nc, mybir)
    nc.compile()
    return nc


def _dedupe_ldweights(nc, mybir):
    """Delete LDWEIGHTS whose exact weights are already resident in the same
    PE-array region (the Tile lowering re-emits one per matmul).  Weights
    persist in the array across matmuls, so back-to-back matmuls on the same
    stationary operand only need the first load.  Region tracking handles
    row/col-tiled partial loads (an overlapping load invalidates)."""
    removed = 0
    for blk in nc.main_func.blocks:
        loaded: dict = {}
        out = []
        for inst in blk.instructions:
            if isinstance(inst, mybir.InstLdweights):
                w = inst.ins[0]
                key = (
                    getattr(w, "memref", None),
                    w.offset,
                    str(w.ap),
                    str(w.dtype),
                    inst.tile_position,
                    inst.tile_size,
                    inst.perf_mode,
                    inst.is_transpose,
                )
                tp = inst.tile_position or (0, 0)
                ts = inst.tile_size or (128, 128)
                region = (tp[0], tp[0] + ts[0], tp[1], tp[1] + ts[1])
                si = inst.sync_info
                no_sync = si is None or (not si.on_wait and not si.on_update)
                if no_sync and loaded.get(region) == key:
                    removed += 1
                    continue
                for r in list(loaded):
                    if not (
                        r[1] <= region[0]
                        or region[1] <= r[0]
                        or r[3] <= region[2]
                        or region[3] <= r[2]
                    ):
                        del loaded[r]
                loaded[region] = key
            out.append(inst)
        blk.instructions[:] = out
    return removed


def _get_module():
    if "nc" not in _CACHE:
        _CACHE["nc"] = _build_module()
    return _CACHE["nc"]


def _mm_np_dtype():
    mmdt = os.environ.get("KERNEL_MM_DTYPE", "bf16")
    if mmdt == "bf16":
        import ml_dtypes

        return ml_dtypes.bfloat16
    if mmdt == "fp16":
        return np.float16
    return np.float32


def _shard_inputs(x, W1, b1, Wh, bh, Wmu, bmu, Wsig, bsig):
    """Build the per-core input maps (host-side layout prep)."""
    NBLK = (MPC + 3) // 4
    mdt = _mm_np_dtype()
    in_maps = []
    for c in range(NCORES):
        mb, half = c % NB, c // NB
        ms = slice(MPC * mb, MPC * (mb + 1))
        xh = x[NHALF * half : NHALF * (half + 1), :]  # [8192, 16]
        xtr = np.ascontiguousarray(xh.T)  # [16, 8192]
        xt_full = np.zeros((128, NHALF), dtype=np.float32)
        for rep in range(4):  # replicas at partition 0/32/64/96 for row tiling
            xt_full[32 * rep : 32 * rep + D, :] = xtr
            xt_full[32 * rep + D, :] = 1.0  # constant row for folded L1 bias

        w1 = W1[ms]  # [25, 128, 16]
        b1c = b1[ms]  # [25, 128]
        w1t = np.zeros((128, NBLK * H), dtype=np.float32)
        for m in range(MPC):
            b, g = m // 4, m % 4
            w1t[32 * g : 32 * g + D, b * H : (b + 1) * H] = w1[m].T
            w1t[32 * g + D, b * H : (b + 1) * H] = b1c[m]  # folded bias row

        wh = Wh[ms]  # [25, 4, 128, 128] (out, in)
        wht = np.ascontiguousarray(
            wh.transpose(3, 0, 1, 2).reshape(H, MPC * NH * H)
        )  # [h_in, (m, i, h_out)]

        # head weights, col-tiled 2x: even models -> array cols 0-63, odd ->
        # cols 64-127; within the 64-col block: col k=m//2 = Wmu, 32+k = Wsig
        whd = np.zeros((H, MPC * 64), dtype=np.float32)
        for m in range(MPC):
            base, k = m * 64, m // 2
            whd[:, base + k] = Wmu[ms][m, 0, :]
            whd[:, base + 32 + k] = Wsig[ms][m, 0, :]

        bhp = np.ascontiguousarray(
            bh[ms].transpose(2, 0, 1).reshape(H, MPC * NH)
        )  # [128, (m, i)]
        bhdp = np.zeros((128, 1), dtype=np.float32)
        bhdp[0:NEV, 0] = bmu[ms][0::2, 0]
        bhdp[32 : 32 + NEV, 0] = bsig[ms][0::2, 0]
        bhdp[64 : 64 + NOD, 0] = bmu[ms][1::2, 0]
        bhdp[96 : 96 + NOD, 0] = bsig[ms][1::2, 0]

        in_maps.append(
            {
                "xt": xt_full.astype(mdt),
                "w1t": w1t.astype(mdt),
                "wht": wht.astype(mdt),
                "whd": whd.astype(mdt),
                "bh": bhp,
                "bhd": bhdp,
            }
        )
    return in_maps


def _run(in_maps, trace=False):
    from concourse.bass_utils import run_bass_kernel_spmd

    nc = _get_module()
    return run_bass_kernel_spmd(
        nc, in_maps, list(range(NCORES)), trace=trace
    )


def kernel(x, W1, b1, Wh, bh, Wmu, bmu, Wsig, bsig):
    args = [
        np.ascontiguousarray(np.asarray(a, dtype=np.float32))
        for a in (x, W1, b1, Wh, bh, Wmu, bmu, Wsig, bsig)
    ]
    in_maps = _shard_inputs(*args)
    res = _run(in_maps, trace=bool(int(os.environ.get("KERNEL_TRACE", "0"))))
    _CACHE["last_results"] = res

    mu = np.empty((M, N), dtype=np.float32)
    sig = np.empty((M, N), dtype=np.float32)
    ev = np.arange(0, MPC, 2)  # storage rows 0..12 hold even models
    od = np.arange(1, MPC, 2)  # storage rows 13..24 hold odd models
    for c in range(NCORES):
        mb, half = c % NB, c // NB
        m0 = MPC * mb
        ns = slice(NHALF * half, NHALF * (half + 1))
        r = res.results[c]
        mu[m0 + ev, ns] = r["mu"][0:NEV]
        mu[m0 + od, ns] = r["mu"][NEV:MPC]
        sig[m0 + ev, ns] = r["sig"][0:NEV]
        sig[m0 + od, ns] = r["sig"][NEV:MPC]
    return (mu.reshape(M, N, O), sig.reshape(M, N, O))
